# revision 48
# baseline (speedup 1.0000x reference)
"""Causal multi-head attention (PBrelax) for TRN2, sharded over 8 NeuronCores.

Sharding: batch (2) x head-group (4 heads each) = 8 shards, one per core.
Each core computes q/k/v projections for its 256 channels, causal attention
in S^T layout (keys on partitions), and a partial output projection; the
host sums the 4 per-batch partials (bf16) and adds bp.

v4 cross-rep pipelining vs v2 (229us/iter -> 214us/iter on HW):
- S^T strips are 512-aligned with a 128-granular causal diagonal (zero
  prefix memset + one triangular mask multiply per diagonal strip) so
  all att@V matmuls are clean full-512 psum accumulation groups.
- the psum pool is hoisted out of the rep loop and qT is double-buffered
  so the NEXT rep's kq0a/kq0b projection passes weave into this rep's
  output-projection tail (PE does not drain at the rep boundary), and
  the next rep's xk/xq piece DMAs issue mid-attention, when the DMA
  engines are otherwise idle; kq1 then pairs with s(0,0) and the v
  projection with s(0,1).
- ACT does exp, plus the outproj PSUM copies in the windows where DVE
  runs the normalize chain (recip -> f32r ones-matmul partition
  broadcast -> row scale) so the reciprocal never gates PE.
- yT written into qT's SBUF space (dead by then); bf16 output, summed
  across the 4 head-group partials on the host in f32.
"""

import numpy as np
import ml_dtypes

import concourse.bass as bass
import concourse.bacc as bacc
import concourse.mybir as mybir
import concourse.tile as tile

BF16 = mybir.dt.bfloat16
F32 = mybir.dt.float32
F32R = mybir.dt.float32r
EXP = mybir.ActivationFunctionType.Exp

B, T_FULL, C, H = 2, 2048, 1024, 16
HD = 64
NH = 4            # heads per core
CS = NH * HD      # 256 channels per core
P = 128
KF = C // P       # 8 contraction chunks
LSCALE = 0.125    # (1/(alpha*sqrt(hd))) * alpha = 1/8
N_CORES = 8


def weave(*gens):
    """Round-robin the generators: one yield-step each per round."""
    gens = [iter(g) for g in gens]
    while gens:
        for g in list(gens):
            try:
                next(g)
            except StopIteration:
                gens.remove(g)


def take(g, n):
    """Yield at most n steps of generator g (g can be resumed later)."""
    for _ in range(n):
        try:
            next(g)
        except StopIteration:
            return
        yield


def build_nc(T=T_FULL, reps=1, dump=False):
    HALF = T // 2
    QTR = T // 4
    NJ = T // P
    nc = bacc.Bacc(target_bir_lowering=False)
    dmp = {}
    if dump:
        for nm in ("kTd", "qTd", "yTd"):
            dmp[nm] = nc.dram_tensor(nm, [P, 2 * T], BF16, kind="ExternalOutput")
        dmp["vd"] = nc.dram_tensor("vd", [P, (T // P) * 260], BF16,
                                   kind="ExternalOutput")

    xq = nc.dram_tensor("xq", [C, T], BF16, kind="ExternalInput")
    xk = nc.dram_tensor("xk", [C, T], BF16, kind="ExternalInput")
    xv = nc.dram_tensor("xv", [C, T], BF16, kind="ExternalInput")
    wq = nc.dram_tensor("wq", [C, CS], BF16, kind="ExternalInput")
    wk = nc.dram_tensor("wk", [C, CS], BF16, kind="ExternalInput")
    wv = nc.dram_tensor("wv", [C, NH * 65], BF16, kind="ExternalInput")
    wp = nc.dram_tensor("wp", [CS, C], BF16, kind="ExternalInput")
    bq2 = nc.dram_tensor("bq2", [P, 2], F32, kind="ExternalInput")
    bk2 = nc.dram_tensor("bk2", [P, 2], F32, kind="ExternalInput")
    bv260 = nc.dram_tensor("bv260", [P, NH * 65], F32, kind="ExternalInput")
    mask2 = nc.dram_tensor("mask2", [P, 2 * P], BF16, kind="ExternalInput")
    ones64 = nc.dram_tensor("ones64", [1, HD], F32R, kind="ExternalInput")
    out = nc.dram_tensor("out", [T, C], BF16, kind="ExternalOutput")

    with tile.TileContext(nc) as tc:
        with tc.tile_pool(name="sb", bufs=1) as sb, \
             tc.tile_pool(name="xp", bufs=4) as xp, \
             tc.tile_pool(name="esA", bufs=1) as esA, \
             tc.tile_pool(name="esB", bufs=1) as esB, \
             tc.tile_pool(name="nrm", bufs=2) as nrm, \
             tc.tile_pool(name="osb", bufs=4) as ob, \
             tc.tile_pool(name="pap", bufs=1, space="PSUM") as pa:

            # ---- weights / constants ----
            wk_m = sb.tile([P, KF * CS], BF16)
            nc.sync.dma_start(wk_m.rearrange("p (c n) -> p c n", c=KF),
                              wk[:, :].rearrange("(c p) n -> p c n", p=P))
            wq_m = sb.tile([P, KF * CS], BF16)
            nc.sync.dma_start(wq_m.rearrange("p (c n) -> p c n", c=KF),
                              wq[:, :].rearrange("(c p) n -> p c n", p=P))
            wv_m = sb.tile([P, KF * NH * 65], BF16)
            wp_s = sb.tile([P, 2 * C], BF16)
            bq_d = sb.tile([P, 2], F32)
            nc.sync.dma_start(bq_d, bq2[:, :])
            bk_d = sb.tile([P, 2], F32)
            nc.sync.dma_start(bk_d, bk2[:, :])
            bv_d = sb.tile([P, NH * 65], F32)
            nc.sync.dma_start(bv_d, bv260[:, :])
            msk_d = sb.tile([P, 2 * P], BF16)
            nc.sync.dma_start(msk_d, mask2[:, :])
            one_s = sb.tile([1, HD], F32R)
            nc.sync.dma_start(one_s, ones64[:, :])

            v_s = sb.tile([P, NJ * 260], BF16, name="vs")

            def alloc_rep():
                # qT shares storage with yT: head h's yT cols are written
                # only after its last S^T read of those qT cols.  bufs=2 so
                # the next rep's projections overlap this rep's outproj.
                # xk/xq tiles are allocated inside head_gen (after the
                # current rep's xv allocs) so the 4-slot x rotation stays
                # in program order.
                st = {}
                st["qT"] = sb.tile([P, 2 * T], BF16, name="qT", tag="qT",
                                   bufs=2)
                st["kT"] = sb.tile([P, 2 * T], BF16, name="kT", tag="kT",
                                   bufs=2)
                return st

            def xdma_gen(st, first=False):
                # x loads for a rep: interleave xk/xq piece DMAs so q
                # passes start early.  In steady state this is woven into
                # the PREVIOUS rep's attention phase, when DMA is idle.
                st["xkm"] = [xp.tile([P, KF * HALF], BF16, tag="x",
                                     name=f"xk{hf}") for hf in range(2)]
                st["xqm"] = [xp.tile([P, KF * HALF], BF16, tag="x",
                                     name=f"xq{hf}") for hf in range(2)]
                for hf in range(2):
                    for kc in range(KF):
                        for xd, xm in ((xk, st["xkm"]), (xq, st["xqm"])):
                            nc.sync.dma_start(
                                xm[hf][:, kc * HALF:(kc + 1) * HALF],
                                xd[kc * P:(kc + 1) * P,
                                   hf * HALF:(hf + 1) * HALF])
                        if not first and kc % 4 == 3:
                            yield
                if first:
                    nc.sync.dma_start(
                        wv_m.rearrange("p (c n) -> p c n", c=KF),
                        wv[:, :].rearrange("(c p) n -> p c n", p=P))
                    nc.sync.dma_start(
                        wp_s.rearrange("p (c n) -> p c n", c=2),
                        wp[:, :].rearrange("(c p) n -> p c n", p=P))

            def proj_pass(w_m, b_t, x_m, out_s, dt, q4):
                # one quarter (512 cols of T) of one dt half
                pk = pa.tile([P, 512], F32, tag="pk", bufs=2, name="pk")
                xh = x_m[q4 // 2]
                c0 = (q4 % 2) * 512
                for kc in range(KF):
                    nc.tensor.matmul(
                        pk, w_m[:, kc * CS + dt * P: kc * CS + dt * P + P],
                        xh[:, kc * HALF + c0: kc * HALF + c0 + 512],
                        start=(kc == 0), stop=(kc == KF - 1))
                nc.vector.tensor_scalar_add(
                    out_s[:, dt * T + q4 * 512: dt * T + (q4 + 1) * 512],
                    pk, b_t[:, dt:dt + 1])
                yield

            def passes_gen(specs):
                for w_m, b_t, x_m, out_s, dt, q4 in specs:
                    yield from proj_pass(w_m, b_t, x_m, out_s, dt, q4)

            def chain(*gs):
                for g in gs:
                    yield from g

            def kq0b_specs(st):
                return [(wk_m, bk_d, st["xkm"], st["kT"], 0, 2),
                        (wk_m, bk_d, st["xkm"], st["kT"], 0, 3),
                        (wq_m, bq_d, st["xqm"], st["qT"], 0, 2),
                        (wq_m, bq_d, st["xqm"], st["qT"], 0, 3)]

            def kq1_specs(st):
                return [(wk_m, bk_d, st["xkm"], st["kT"], 1, 0),
                        (wk_m, bk_d, st["xkm"], st["kT"], 1, 1),
                        (wq_m, bq_d, st["xqm"], st["qT"], 1, 0),
                        (wq_m, bq_d, st["xqm"], st["qT"], 1, 1),
                        (wk_m, bk_d, st["xkm"], st["kT"], 1, 2),
                        (wk_m, bk_d, st["xkm"], st["kT"], 1, 3),
                        (wq_m, bq_d, st["xqm"], st["qT"], 1, 2),
                        (wq_m, bq_d, st["xqm"], st["qT"], 1, 3)]

            def pair_gen(st):
                # the first two kq quarter-pairs of a rep: k and q dt0
                # quarter passes interleaved at kc granularity
                for q4 in range(2):
                    pkk = pa.tile([P, 512], F32, tag="pk", bufs=2, name="pkk")
                    pkq = pa.tile([P, 512], F32, tag="pk", bufs=2, name="pkq")
                    c0 = (q4 % 2) * 512
                    for kc in range(KF):
                        for w_m, x_m, pk in ((wk_m, st["xkm"], pkk),
                                             (wq_m, st["xqm"], pkq)):
                            nc.tensor.matmul(
                                pk, w_m[:, kc * CS: kc * CS + P],
                                x_m[q4 // 2][:, kc * HALF + c0:
                                             kc * HALF + c0 + 512],
                                start=(kc == 0), stop=(kc == KF - 1))
                    for b_t, out_s, pk in ((bk_d, st["kT"], pkk),
                                           (bq_d, st["qT"], pkq)):
                        nc.vector.tensor_scalar_add(
                            out_s[:, q4 * 512:(q4 + 1) * 512],
                            pk, b_t[:, 0:1])
                    yield

            def run_rep(st, nxt_st, rep_last):
                qT_s = st["qT"]
                kT_s = st["kT"]
                xkm, xqm = st["xkm"], st["xqm"]
                yT_s = qT_s

                def load_half(xd, hf, name):
                    xm = xp.tile([P, KF * HALF], BF16, tag="x", name=name)
                    for kc in range(KF):
                        nc.sync.dma_start(
                            xm[:, kc * HALF:(kc + 1) * HALF],
                            xd[kc * P:(kc + 1) * P, hf * HALF:(hf + 1) * HALF])
                    return xm

                # ---- S^T strips + exp for one (head, query-half) unit ----
                # es strips are 512-aligned with a <=384-col zero prefix
                # (Pool memset - the engine is otherwise idle) so every
                # att@V matmul is a clean full-512 psum accumulation group.
                def s_gen(h, qh, es_pool):
                    ht, hr = h // 2, (h % 2) * 64
                    base = qh * HALF
                    strips = []
                    for jc in range(min(NJ, 8 * (qh + 1))):
                        s0 = max(jc * P, base)
                        w = base + HALF - s0
                        diag = jc * P >= base
                        ba = (s0 // 512) * 512
                        pre = s0 - ba
                        ps = pa.tile([P, HALF], F32, tag="ps", bufs=2,
                                     name="pst")
                        q0 = 0
                        while q0 < w:
                            qw = min(512 - q0 % 512, w - q0)
                            nc.tensor.matmul(
                                ps[:, q0:q0 + qw],
                                kT_s[hr:hr + 64,
                                     ht * T + jc * P: ht * T + (jc + 1) * P],
                                qT_s[hr:hr + 64,
                                     ht * T + s0 + q0: ht * T + s0 + q0 + qw],
                                start=True, stop=True)
                            q0 += qw
                        tag, tw = ("es", HALF) if pre + w > 512 \
                            else ("esn", 512)
                        es = es_pool.tile([P, tw], BF16, tag=tag,
                                          bufs=14 if tag == "es" else 8,
                                          name="es")
                        if pre:
                            nc.gpsimd.memset(es[:, 0:pre], 0.0)
                        nc.scalar.activation(es[:, pre:pre + w],
                                             ps[:, 0:w], EXP, scale=LSCALE)
                        if diag:
                            # zero the sub-diagonal of the 128-wide
                            # diagonal block (tri mask, bf16 2x on DVE)
                            nc.vector.tensor_mul(
                                es[:, pre:pre + P], es[:, pre:pre + P],
                                msk_d[:, 0:P])
                        strips.append((jc, ba, pre, es))
                        yield
                    sdict[(h, qh)] = strips

                # ---- normalize one (head, quarter): yT = py/denom ----
                def norm_qt(h, qt, py):
                    ht, hr = h // 2, (h % 2) * 64
                    lo = qt * QTR
                    rhr = nrm.tile([1, QTR], F32R, tag="rhr", name="rhr")
                    with nc.allow_low_precision(reason="f32r row-scale"):
                        nc.vector.reciprocal(rhr, py[64:65, :])
                    rb = pa.tile([P, 512], F32, tag="pk", bufs=2, name="rb")
                    nc.tensor.matmul(rb[0:64, 0:QTR], one_s, rhr,
                                     start=True, stop=True)
                    rbs = nrm.tile([HD, QTR], F32, tag="rbs", name="rbs")
                    nc.vector.tensor_copy(rbs, rb[0:64, 0:QTR])
                    nc.vector.tensor_mul(
                        yT_s[hr:hr + 64, ht * T + lo: ht * T + lo + QTR],
                        py[0:64, :], rbs)

                # ---- att@V for one unit; norms inline per quarter ----
                def yt_gen(h, qh):
                    strips = sdict[(h, qh)]
                    for qt in (2 * qh, 2 * qh + 1):
                        lo = qt * QTR
                        hi = lo + QTR
                        py = pa.tile([65, QTR], F32, tag="py", bufs=2,
                                     name="py")
                        nmm = 0
                        for jc, ba, pre, es in strips:
                            if ba + pre >= hi:
                                continue
                            nc.tensor.matmul(
                                py,
                                v_s[:, jc * 260 + h * 65:
                                    jc * 260 + h * 65 + 65],
                                es[:, lo - ba:hi - ba],
                                start=(jc == 0),
                                stop=(jc == hi // P - 1))
                            nmm += 1
                            if nmm % 4 == 0:
                                yield
                        yield
                        norm_qt(h, qt, py)
                        yield

                # ---- output projection for one quarter of T rows ----
                def outproj_gen(qt, copies_on_act=False):
                    for it in range(qt * QTR // P, (qt + 1) * QTR // P):
                        for nn in range(2):
                            pot = pa.tile([P, 512], F32, tag="pk", bufs=2,
                                          name="pot")
                            for ct in range(2):
                                nc.tensor.matmul(
                                    pot,
                                    yT_s[:, ct * T + it * P:
                                         ct * T + (it + 1) * P],
                                    wp_s[:, ct * C + nn * 512:
                                         ct * C + nn * 512 + 512],
                                    start=(ct == 0), stop=(ct == 1))
                            ot = ob.tile([P, 512], BF16, tag="ot", name="ot")
                            if nn == 0 and not copies_on_act:
                                nc.vector.tensor_copy(ot, pot)
                            else:
                                nc.scalar.copy(ot, pot)
                            nc.sync.dma_start(
                                out[it * P:(it + 1) * P,
                                    nn * 512:(nn + 1) * 512], ot)
                        yield

                # ---- schedule ----
                sdict = {}

                def v_gen(xvm):
                    for jt in range(NJ):
                        pv = pa.tile([P, 512], F32, tag="pk", bufs=2,
                                     name="pv")
                        xh = xvm[jt // 8]
                        t0 = (jt % 8) * P
                        for kc in range(KF):
                            nc.tensor.matmul(
                                pv[:, 0:NH * 65],
                                xh[:, kc * HALF + t0: kc * HALF + t0 + P],
                                wv_m[:, kc * NH * 65:(kc + 1) * NH * 65],
                                start=(kc == 0), stop=(kc == KF - 1))
                        nc.vector.tensor_add(
                            v_s[:, jt * 260:(jt + 1) * 260],
                            pv[:, 0:NH * 65], bv_d)
                        yield

                # unit order (h,qh): each head's big qh1-exp unit follows
                # its qh0 unit so ACT is fed heavy work early; yt runs
                # one unit behind S; norms inline one quarter behind yt.
                # kq0a+kq0b were emitted in the previous rep's tail, so
                # kq1 pairs with s(0,0) and v-proj with s(0,1); the next
                # rep's x DMAs issue mid-attention (DMA is idle there).
                weave(passes_gen(kq1_specs(st)), s_gen(0, 0, esA))
                if dump:
                    nc.sync.dma_start(dmp["kTd"][:, :], kT_s)
                    nc.sync.dma_start(dmp["qTd"][:, :], qT_s)
                xvm = [load_half(xv, hf, f"xv{hf}") for hf in range(2)]
                weave(v_gen(xvm), s_gen(0, 1, esB))
                weave(s_gen(1, 1, esA), yt_gen(0, 0))
                if nxt_st is None:
                    weave(s_gen(1, 0, esB), yt_gen(0, 1))
                    weave(s_gen(2, 1, esA), yt_gen(1, 1))
                else:
                    weave(s_gen(1, 0, esB), yt_gen(0, 1), xdma_gen(nxt_st))
                    weave(s_gen(2, 1, esA), yt_gen(1, 1))
                weave(s_gen(3, 1, esB), yt_gen(1, 0))
                weave(s_gen(2, 0, esA), yt_gen(2, 1))
                weave(s_gen(3, 0, esB), yt_gen(3, 1))
                # copies on ACT here: exp is finished, and DVE is busy
                # with the yt(2,0)/yt(3,0) normalize chains whose recip
                # output gates the rb broadcast matmuls on PE
                weave(yt_gen(2, 0), outproj_gen(2, True))
                weave(yt_gen(3, 0), outproj_gen(3, True))
                if nxt_st is None:
                    weave(outproj_gen(0, True), outproj_gen(1, True))
                else:
                    weave(outproj_gen(0, True), outproj_gen(1, True),
                          chain(pair_gen(nxt_st),
                                passes_gen(kq0b_specs(nxt_st))))
                if dump and rep_last:
                    nc.sync.dma_start(dmp["vd"][:, :], v_s)
                    nc.sync.dma_start(dmp["yTd"][:, :], yT_s)

            st = alloc_rep()
            for _ in xdma_gen(st, first=True):
                pass
            for _ in chain(pair_gen(st), passes_gen(kq0b_specs(st))):
                pass
            for rep in range(reps):
                nxt_st = alloc_rep() if rep + 1 < reps else None
                run_rep(st, nxt_st, rep == reps - 1)
                st = nxt_st

    return nc


def make_core_inputs(query, key, value, Wq, bq, Wk, bk, Wv, bv, Wp, T=T_FULL):
    """Host-side shard prep. Returns list of 8 in_maps (bf16 numpy)."""
    bf = ml_dtypes.bfloat16
    query = np.asarray(query, np.float32)
    key = np.asarray(key, np.float32)
    value = np.asarray(value, np.float32)
    Wq, bq = np.asarray(Wq, np.float32), np.asarray(bq, np.float32)
    Wk, bk = np.asarray(Wk, np.float32), np.asarray(bk, np.float32)
    Wv, bv = np.asarray(Wv, np.float32), np.asarray(bv, np.float32)
    Wp = np.asarray(Wp, np.float32)

    kk = np.arange(P)[:, None]   # tk within diagonal block (partition)
    mm = np.arange(P)[None, :]   # tq within diagonal block (free)
    tri = (mm >= kk).astype(np.float32).astype(bf)
    mask2 = np.concatenate([tri, np.zeros((P, P), bf)], axis=1)
    ones64 = np.ones((1, HD), np.float32)

    xT = {}
    for nm, x in (("q", query), ("k", key), ("v", value)):
        for b in range(B):
            xT[nm, b] = np.ascontiguousarray(x[b].T).astype(bf)

    in_maps = []
    for core in range(N_CORES):
        b, g = core // 4, core % 4
        hs = slice(g * CS, (g + 1) * CS)
        wv_p = np.zeros((C, NH * 65), np.float32)
        bv_p = np.zeros((P, NH * 65), np.float32)
        wv_h = Wv[:, hs]
        for h in range(NH):
            wv_p[:, h * 65:h * 65 + 64] = wv_h[:, h * 64:(h + 1) * 64]
            bv_p[:, h * 65:h * 65 + 64] = bv[hs][h * 64:(h + 1) * 64][None, :]
            bv_p[:, h * 65 + 64] = 1.0
        in_maps.append(dict(
            xq=xT["q", b], xk=xT["k", b], xv=xT["v", b],
            wq=Wq[:, hs].astype(bf), wk=Wk[:, hs].astype(bf),
            wv=wv_p.astype(bf), wp=Wp[hs, :].astype(bf),
            bq2=np.ascontiguousarray(bq[hs].reshape(2, P).T),
            bk2=np.ascontiguousarray(bk[hs].reshape(2, P).T),
            bv260=bv_p, mask2=mask2, ones64=ones64))
    return in_maps


_NC = None
TRACE = False
LAST = None


def kernel(query, key, value, att_mask, Wq, bq, Wk, bk, Wv, bv, Wp, bp):
    from concourse.bass_utils import run_bass_kernel_spmd
    global _NC, LAST
    if _NC is None:
        _NC = build_nc()
        _NC.finalize()
    in_maps = make_core_inputs(query, key, value, Wq, bq, Wk, bk, Wv, bv, Wp)
    res = run_bass_kernel_spmd(_NC, in_maps, core_ids=list(range(N_CORES)),
                               trace=TRACE)
    LAST = res
    full = np.zeros((B, T_FULL, C), np.float32)
    for core in range(N_CORES):
        full[core // 4] += np.asarray(res.results[core]["out"], np.float32)
    full += np.asarray(bp, np.float32)[None, None, :]
    return full
